# revision 1
# baseline (speedup 1.0000x reference)
"""Trainium2 Bass kernel for an attention block (B=8, T=2048, D=K=V=1024).

Reference math (per batch element, sharded one per NeuronCore):
    Q = x @ Wq.T + bq ; K = x @ Wk.T + bk ; V = x @ Wv.T + bv
    logits[t,s] = Q[t] . K[s],  masked -inf for s > t (strict upper tri)
    probs = softmax(logits, axis=t) / sqrt(1024)     # softmax over QUERY axis
    out = x + probs @ V

Implementation notes:
  - Everything is computed in a transposed layout: QT/KT are [k, t] (k on
    partitions) so logitsT = [s, t] comes straight out of the PE, and the
    softmax reduction (over t) is a free-axis reduction.
  - softmax over t for fixed s:  P[s,t] = exp(l[s,t]);  Z[s] = sum_t P[s,t];
    read[t,v] = sum_s P[s,t] * V[s,v] / (32 * Z[s]).  No max subtraction is
    needed: logits are ~N(0,13^2), max |l| < ~75 so exp stays inside fp32.
  - Matmuls run as float32r (full PE rate at free dim 512). P and V are
    stored bf16 for the PV matmul (also full rate).
  - Causal structure skips fully-masked tiles; diagonal 128x512 tiles get an
    additive -1e30 staircase mask.
"""

import time

import numpy as np

import concourse.bass as bass
import concourse.bacc as bacc
import concourse.mybir as mybir
import concourse.tile as tile
from concourse.bass_utils import run_bass_kernel_spmd
from concourse.masks import make_identity

F32 = mybir.dt.float32
F32R = mybir.dt.float32r
BF16 = mybir.dt.bfloat16
AF = mybir.ActivationFunctionType

P = 128          # partitions
T = 2048         # sequence length
D = 1024         # model dim
TB = 512         # t-block width
NTB = T // TB    # 4 t-blocks
DK = D // P      # 8 contraction subtiles
KO = D // P      # 8 k output tiles
SV = T // P      # 16 s tiles
NEG = -1.0e30


def _transpose_weight(nc, tc, pools, w_ap, dst):
    """Transpose a [1024, 1024] DRAM weight into dst SBUF tile [128, 8, 1024]
    laid out as dst[d_inner, d_outer, k]."""
    wnat_pool, psum_t, identity = pools
    for kt in range(8):
        wnat = wnat_pool.tile([P, D], F32R, name="wnat", tag="wnat")
        eng_a = nc.gpsimd if kt % 2 == 0 else nc.sync
        eng_b = nc.sync if kt % 2 == 0 else nc.gpsimd
        eng_a.dma_start(out=wnat[:P // 2, :],
                        in_=w_ap[kt * P:kt * P + P // 2, :].bitcast(F32R))
        eng_b.dma_start(out=wnat[P // 2:, :],
                        in_=w_ap[kt * P + P // 2:(kt + 1) * P, :].bitcast(F32R))
        for dk in range(DK):
            pt = psum_t.tile([P, P], F32R, name="pt", tag="pt")
            nc.tensor.transpose(
                pt,
                wnat[:, dk * P:(dk + 1) * P],
                identity,
            )
            nc.vector.tensor_copy(out=dst[:, dk, kt * P:(kt + 1) * P], in_=pt)


def _build_nc():
    nc = bacc.Bacc("TRN2", target_bir_lowering=False, debug=False, num_devices=8)

    x = nc.dram_tensor("x", [T, D], F32, kind="ExternalInput").ap()
    Wq = nc.dram_tensor("Wq", [D, D], F32, kind="ExternalInput").ap()
    bq = nc.dram_tensor("bq", [D], F32, kind="ExternalInput").ap()
    Wk = nc.dram_tensor("Wk", [D, D], F32, kind="ExternalInput").ap()
    bk = nc.dram_tensor("bk", [D], F32, kind="ExternalInput").ap()
    Wv = nc.dram_tensor("Wv", [D, D], F32, kind="ExternalInput").ap()
    bv = nc.dram_tensor("bv", [D], F32, kind="ExternalInput").ap()
    out = nc.dram_tensor("out", [T, D], F32, kind="ExternalOutput").ap()

    with tile.TileContext(nc) as tc:
        _kernel_body(nc, tc, x, Wq, bq, Wk, bk, Wv, bv, out)

    nc.compile()
    return nc


def _kernel_body(nc, tc, x, Wq, bq, Wk, bk, Wv, bv, out):
    from contextlib import ExitStack

    ctx = ExitStack()
    with ctx:
        consts = ctx.enter_context(tc.tile_pool(name="consts", bufs=1))
        wpool = ctx.enter_context(tc.tile_pool(name="wpool", bufs=2))
        ktpool = ctx.enter_context(tc.tile_pool(name="ktpool", bufs=1))
        dram = ctx.enter_context(tc.tile_pool(name="dram", bufs=1, space="DRAM"))
        psum_t = ctx.enter_context(tc.tile_pool(name="psum_t", bufs=3, space="PSUM"))
        psum_mm = ctx.enter_context(tc.tile_pool(name="psum_mm", bufs=5, space="PSUM"))

        # ---- constants ----
        # identity first: it gates every PE transpose at kernel start
        id_f32 = consts.tile([P, P], F32, name="id_f32")
        make_identity(nc, id_f32)
        identity = consts.tile([P, P], F32R, name="identity")
        nc.vector.tensor_copy(out=identity, in_=id_f32)

        # staircase masks for the 4 diagonal positions of a [128 s, 512 t]
        # tile with offset o = s0 - t0 in {0,128,256,384}: valid iff f >= p + o
        # (masks / biases / Z built after the first x-transposes are queued,
        # so they don't block the gpsimd DMA queue at kernel start)
        rtile = consts.tile([P, SV], F32, name="rtile")

        # persistent KT [k_inner, k_outer, s]
        KT = ktpool.tile([P, KO, T], F32R, name="KT", tag="big")

        # DRAM scratch
        xT_dram = dram.tile([NTB, P, DK, TB], F32R, name="xT_dram")
        P_dram = dram.tile([P, SV, SV, P], BF16, name="P_dram")

        with (
            tc.tile_pool(name="wnat", bufs=3) as wnat_pool,
            tc.tile_pool(name="xnat", bufs=3) as xnat_pool,
            tc.tile_pool(name="xtp", bufs=2) as xtp,
            tc.tile_pool(name="qtp", bufs=1) as qtp,
            tc.tile_pool(name="pstage", bufs=2) as pstage,
        ):
            tpools = (wnat_pool, psum_t, identity)

            def make_xT_blk(j):
                xT_blk = xtp.tile([P, DK, TB], F32R, name="xT_blk", tag="xT")
                for ts_ in range(TB // P):
                    t0 = j * TB + ts_ * P
                    xnat = xnat_pool.tile([P, D], F32R, name="xnat", tag="xnat")
                    # split by partition: cuts the per-transfer descriptor
                    # count (DMA engines are descriptor-rate-bound) and runs
                    # the pieces on parallel HW queues. gpsimd ring only
                    # (sync is busy with P/xT writes that wait on compute).
                    # The very first tiles use quarter-splits so the first PE
                    # transpose can start as early as possible.
                    nsplit = 4 if (j == 0 and ts_ < 2) else 2
                    step = P // nsplit
                    for q in range(nsplit):
                        nc.gpsimd.dma_start(
                            out=xnat[q * step:(q + 1) * step, :],
                            in_=x[t0 + q * step:t0 + (q + 1) * step, :].bitcast(F32R))
                    for dk in range(DK):
                        pt = psum_t.tile([P, P], F32R, name="pt", tag="pt")
                        nc.tensor.transpose(
                            pt,
                            xnat[:, dk * P:(dk + 1) * P],
                            identity,
                        )
                        nc.vector.tensor_copy(
                            out=xT_blk[:, dk, ts_ * P:(ts_ + 1) * P], in_=pt
                        )
                nc.sync.dma_start(out=xT_dram[j], in_=xT_blk)
                return xT_blk

            # j=0's x transposes run first: x tiles arrive long before the
            # full weight matrices, so this keeps the PE busy from ~2us.
            xT_first = make_xT_blk(0)

            # one sliding mask [128, 896]: valid (0.0) iff g >= p + 384,
            # else -1e30. mask for diagonal offset oi*128 is the slice
            # [384-128*oi : 896-128*oi].
            mask_base = consts.tile([P, TB + 3 * P], BF16, name="mask_base")
            nc.gpsimd.memset(mask_base, 0.0)
            nc.gpsimd.affine_select(
                out=mask_base, in_=mask_base,
                compare_op=mybir.AluOpType.is_ge,
                fill=NEG,
                base=-(3 * P),
                pattern=[[1, TB + 3 * P]],
                channel_multiplier=-1,
            )
            masks = [mask_base[:, 3 * P - oi * P: 3 * P - oi * P + TB]
                     for oi in range(4)]

            # biases: bq/bk striped [128, 8] (per-partition, k-major);
            # bv broadcast to all partitions [128, 1024]
            bq_sb = consts.tile([P, KO], F32, name="bq_sb")
            nc.sync.dma_start(out=bq_sb, in_=bq.rearrange("(o p) -> p o", p=P))
            bk_sb = consts.tile([P, KO], F32, name="bk_sb")
            nc.sync.dma_start(out=bk_sb, in_=bk.rearrange("(o p) -> p o", p=P))
            bv_sb = consts.tile([P, D], BF16, name="bv_sb")
            bv_bcast = bass.AP(tensor=bv.tensor, offset=bv.offset,
                               ap=[[0, P], [1, D]])
            nc.gpsimd.dma_start(out=bv_sb, in_=bv_bcast)

            Zacc = consts.tile([P, SV, NTB], F32, name="Zacc")
            nc.vector.memset(Zacc, 0.0)

            # ---- phase 1: weight transposes for Q, K ----
            WqT = wpool.tile([P, DK, D], F32R, name="WqT", tag="W")
            _transpose_weight(nc, tc, tpools, Wq, WqT)
            WkT = wpool.tile([P, DK, D], F32R, name="WkT", tag="W")
            _transpose_weight(nc, tc, tpools, Wk, WkT)

            # ---- phase 2: fused x-transpose + QT/KT + logits + exp sweep ----
            xT_next = xT_first
            for j in range(NTB):
                xT_blk = xT_next

                # QT block [k_inner, k_outer, t(512)]
                qt_blk = qtp.tile([P, KO, TB], F32R, name="qt_blk", tag="qt")
                for ko in range(KO):
                    ps = psum_mm.tile([P, TB], F32, name="ps_q", tag="mm")
                    for dk in range(DK):
                        nc.tensor.matmul(
                            ps,
                            lhsT=WqT[:, dk, ko * P:(ko + 1) * P],
                            rhs=xT_blk[:, dk, :],
                            start=(dk == 0),
                            stop=(dk == DK - 1),
                        )
                    nc.scalar.activation(
                        qt_blk[:, ko, :], ps, AF.Identity, bias=bq_sb[:, ko:ko + 1]
                    )

                # KT block
                for ko in range(KO):
                    ps = psum_mm.tile([P, TB], F32, name="ps_k", tag="mm")
                    for dk in range(DK):
                        nc.tensor.matmul(
                            ps,
                            lhsT=WkT[:, dk, ko * P:(ko + 1) * P],
                            rhs=xT_blk[:, dk, :],
                            start=(dk == 0),
                            stop=(dk == DK - 1),
                        )
                    nc.scalar.activation(
                        KT[:, ko, j * TB:(j + 1) * TB], ps, AF.Identity,
                        bias=bk_sb[:, ko:ko + 1],
                    )

                # next block's x transposes are emitted mid-block so the PE
                # reaches them long after their xnat DMAs were issued (no
                # boundary stall), hidden between logits tiles
                logits_order = list(range(4 * (j + 1)))
                split = max(0, len(logits_order) - 4)
                for sv in logits_order[:split]:
                    ps = psum_mm.tile([P, TB], F32, name="ps_l", tag="mm")
                    for ko in range(KO):
                        nc.tensor.matmul(
                            ps,
                            lhsT=KT[:, ko, sv * P:(sv + 1) * P],
                            rhs=qt_blk[:, ko, :],
                            start=(ko == 0),
                            stop=(ko == KO - 1),
                        )
                    oi = sv - 4 * j
                    if oi >= 0:
                        nc.vector.tensor_add(out=ps, in0=ps, in1=masks[oi])
                    pst = pstage.tile([P, TB], BF16, name="pst", tag="pst")
                    nc.scalar.activation(
                        pst, ps, AF.Exp, accum_out=Zacc[:, sv, j:j + 1]
                    )
                    nc.sync.dma_start(
                        out=P_dram[:, 4 * j:4 * j + 4, sv, :],
                        in_=pst.rearrange("p (i t) -> p i t", i=4),
                    )
                if j + 1 < NTB:
                    xT_next = make_xT_blk(j + 1)
                for sv in logits_order[split:]:
                    ps = psum_mm.tile([P, TB], F32, name="ps_l", tag="mm")
                    for ko in range(KO):
                        nc.tensor.matmul(
                            ps,
                            lhsT=KT[:, ko, sv * P:(sv + 1) * P],
                            rhs=qt_blk[:, ko, :],
                            start=(ko == 0),
                            stop=(ko == KO - 1),
                        )
                    oi = sv - 4 * j
                    if oi >= 0:
                        nc.vector.tensor_add(out=ps, in0=ps, in1=masks[oi])
                    pst = pstage.tile([P, TB], BF16, name="pst", tag="pst")
                    nc.scalar.activation(
                        pst, ps, AF.Exp, accum_out=Zacc[:, sv, j:j + 1]
                    )
                    nc.sync.dma_start(
                        out=P_dram[:, 4 * j:4 * j + 4, sv, :],
                        in_=pst.rearrange("p (i t) -> p i t", i=4),
                    )

            # ---- Z -> R = 1/(32 Z) ----
            zsum = consts.tile([P, SV], F32, name="zsum")
            nc.vector.reduce_sum(out=zsum, in_=Zacc, axis=mybir.AxisListType.X)
            nc.vector.reciprocal(rtile, zsum)
            nc.vector.tensor_scalar_mul(rtile, rtile, 1.0 / 32.0)

            # ---- phase 3: V' = (x @ Wv.T + bv) / (32 Z), written straight
            # into Vp (which reuses KT's SBUF slot, free after phase 2) ----
            Vp = ktpool.tile([P, SV, D], BF16, name="Vp", tag="big")
            WvT = wpool.tile([P, DK, D], F32R, name="WvT", tag="W")
            _transpose_weight(nc, tc, tpools, Wv, WvT)
            for j in range(NTB):
                xT_blk2 = xtp.tile([P, DK, TB], F32R, name="xT_blk2", tag="xT")
                nc.sync.dma_start(out=xT_blk2, in_=xT_dram[j])
                for si in range(TB // P):
                    sv = j * 4 + si
                    for h in range(D // TB):
                        ps = psum_mm.tile([P, TB], F32, name="ps_v", tag="mm")
                        for dk in range(DK):
                            nc.tensor.matmul(
                                ps,
                                lhsT=xT_blk2[:, dk, si * P:(si + 1) * P],
                                rhs=WvT[:, dk, h * TB:(h + 1) * TB],
                                start=(dk == 0),
                                stop=(dk == DK - 1),
                            )
                        nc.vector.tensor_add(
                            out=Vp[:, sv, h * TB:(h + 1) * TB],
                            in0=ps,
                            in1=bv_sb[:, h * TB:(h + 1) * TB],
                        )
                        nc.vector.tensor_scalar_mul(
                            Vp[:, sv, h * TB:(h + 1) * TB],
                            Vp[:, sv, h * TB:(h + 1) * TB],
                            rtile[:, sv:sv + 1],
                        )

        # ---- phase 4: read = P^T . V', out = x + read ----
        with (
            tc.tile_pool(name="pcol", bufs=3) as pcol_pool,
            tc.tile_pool(name="ost", bufs=2) as ost_pool,
            tc.tile_pool(name="xres", bufs=2) as xres_pool,
        ):
            for i in range(SV):
                pcol = pcol_pool.tile([P, SV, P], BF16, name="pcol", tag="pcol")
                nc.gpsimd.dma_start(
                    out=pcol[:, 0:i + 1, :], in_=P_dram[:, i, 0:i + 1, :]
                )
                xres = xres_pool.tile([P, D], F32, name="xres", tag="xres")
                nc.gpsimd.dma_start(out=xres, in_=x[i * P:(i + 1) * P, :])
                ost = ost_pool.tile([P, D], F32, name="ost", tag="ost")
                for h in range(D // TB):
                    ps = psum_mm.tile([P, TB], F32, name="ps_o", tag="mm")
                    for svv in range(i + 1):
                        nc.tensor.matmul(
                            ps,
                            lhsT=pcol[:, svv, :],
                            rhs=Vp[:, svv, h * TB:(h + 1) * TB],
                            start=(svv == 0),
                            stop=(svv == i),
                        )
                    nc.vector.tensor_add(
                        out=ost[:, h * TB:(h + 1) * TB],
                        in0=ps,
                        in1=xres[:, h * TB:(h + 1) * TB],
                    )
                nc.sync.dma_start(out=out[i * P:(i + 1) * P, :], in_=ost)


_NC_CACHE = None


def _get_nc():
    global _NC_CACHE
    if _NC_CACHE is None:
        _NC_CACHE = _build_nc()
    return _NC_CACHE


def kernel(minibatch, Wq, bq, Wk, bk, Wv, bv):
    minibatch = np.asarray(minibatch, dtype=np.float32)
    Wq = np.asarray(Wq, dtype=np.float32)
    bq = np.asarray(bq, dtype=np.float32)
    Wk = np.asarray(Wk, dtype=np.float32)
    bk = np.asarray(bk, dtype=np.float32)
    Wv = np.asarray(Wv, dtype=np.float32)
    bv = np.asarray(bv, dtype=np.float32)

    nc = _get_nc()
    B = minibatch.shape[0]
    in_maps = [
        {
            "x": np.ascontiguousarray(minibatch[i]),
            "Wq": Wq, "bq": bq, "Wk": Wk, "bk": bk, "Wv": Wv, "bv": bv,
        }
        for i in range(B)
    ]
    last_err = None
    for _attempt in range(3):
        try:
            res = run_bass_kernel_spmd(nc, in_maps, core_ids=list(range(B)))
            break
        except Exception as e:  # transient device errors (e.g. NRT_EXEC_UNIT_UNRECOVERABLE)
            last_err = e
            time.sleep(2.0)
    else:
        raise last_err
    return np.stack([res.results[i]["out"] for i in range(B)], axis=0)



# revision 7
# speedup vs baseline: 1.4569x; 1.4569x over previous
"""Trainium2 Bass kernel for an attention block (B=8, T=2048, D=K=V=1024).

Reference math (per batch element, sharded one per NeuronCore):
    Q = x @ Wq.T + bq ; K = x @ Wk.T + bk ; V = x @ Wv.T + bv
    logits[t,s] = Q[t] . K[s],  masked -inf for s > t (strict upper tri)
    probs = softmax(logits, axis=t) / sqrt(1024)     # softmax over QUERY axis
    out = x + probs @ V

v2 implementation: all matmuls run in fp8 (e4m3) DoubleRow perf mode (two
128-row contraction slices per instruction, 2x PE rate):
  - Weights are scaled x32 into e4m3's sweet spot at load, transposed on the
    PE in bf16, stored fp8. x is transposed bf16 -> fp8 (std 1, no scale).
  - QT/KT come out of PSUM through the scalar engine (bias add + 1/32 scale)
    straight into fp8; logits = KT.T @ QT also runs fp8 DoubleRow.
    Numerics (simulated offline): rel_err ~4e-3 vs 2e-2 tolerance.
  - P = exp(logits) is kept in SBUF in bf16 (causal tiles only, 40 x
    [128,512]); no max subtraction needed (|logits| < ~80 fits bf16 range).
  - softmax over t for fixed s: Z[s] = sum_t P[s,t]; during the last t-block's
    logits sweep, R[s]=1/Z finalizes per s-tile and P tiles convert to fp8
    normalized (P*R in [0,1]); the PV matmuls + residual epilogue interleave
    behind it, so phase 4 has no serial tail.
  - Vp = 32*(V+bv) in fp8 (std ~20, fits e4m3 max 240); the extra 32 and the
    softmax 1/32 fold into the epilogue scale 1/1024.
  - Odd-length PV contractions round up to a DoubleRow pair: the extra s-block
    is always a fully-masked (all-zero) P tile, so it contributes nothing.
"""

import time

import numpy as np

import concourse.bass as bass
import concourse.bacc as bacc
import concourse.mybir as mybir
import concourse.tile as tile
from concourse.bass_utils import run_bass_kernel_spmd
from concourse.masks import make_identity

F32 = mybir.dt.float32
BF16 = mybir.dt.bfloat16
FP8 = mybir.dt.float8e4
AF = mybir.ActivationFunctionType
DR = mybir.MatmulPerfMode.DoubleRow

P = 128          # partitions
T = 2048         # sequence length
D = 1024         # model dim
TB = 512         # t-block width
NTB = T // TB    # 4 t-blocks
KO = D // P      # 8 k output tiles
DK = D // P      # 8 contraction subtiles
SV = T // P      # 16 s tiles
NEG = -1.0e30
WS = 32.0        # weight quantization scale (W*32 ~ std 0.64 in e4m3)
PBASE = [0, 4, 12, 24]   # flat index base of j's causal P tiles (4(j+1) each)


def _build_nc():
    nc = bacc.Bacc("TRN2", target_bir_lowering=False, debug=False, num_devices=8)

    x = nc.dram_tensor("x", [T, D], F32, kind="ExternalInput").ap()
    Wq = nc.dram_tensor("Wq", [D, D], F32, kind="ExternalInput").ap()
    bq = nc.dram_tensor("bq", [D], F32, kind="ExternalInput").ap()
    Wk = nc.dram_tensor("Wk", [D, D], F32, kind="ExternalInput").ap()
    bk = nc.dram_tensor("bk", [D], F32, kind="ExternalInput").ap()
    Wv = nc.dram_tensor("Wv", [D, D], F32, kind="ExternalInput").ap()
    bv = nc.dram_tensor("bv", [D], F32, kind="ExternalInput").ap()
    out = nc.dram_tensor("out", [T, D], F32, kind="ExternalOutput").ap()

    with tile.TileContext(nc) as tc:
        _kernel_body(nc, tc, x, Wq, bq, Wk, bk, Wv, bv, out)

    nc.compile()
    return nc


def _kernel_body(nc, tc, x, Wq, bq, Wk, bk, Wv, bv, out):
    from contextlib import ExitStack

    ctx = ExitStack()
    with ctx:
        consts = ctx.enter_context(tc.tile_pool(name="consts", bufs=1))
        wt8p = ctx.enter_context(tc.tile_pool(name="wt8", bufs=1))
        xt8p = ctx.enter_context(tc.tile_pool(name="xt8", bufs=1))
        kt8p = ctx.enter_context(tc.tile_pool(name="kt8", bufs=1))
        vp8p = ctx.enter_context(tc.tile_pool(name="vp8", bufs=1))
        pbigp = ctx.enter_context(tc.tile_pool(name="pbig", bufs=1))
        pq8p = ctx.enter_context(tc.tile_pool(name="pq8", bufs=1))
        qt8p = ctx.enter_context(tc.tile_pool(name="qt8", bufs=2))
        xnatp = ctx.enter_context(tc.tile_pool(name="xnat", bufs=3))
        xbfp = ctx.enter_context(tc.tile_pool(name="xbf", bufs=2))
        wnatp = ctx.enter_context(tc.tile_pool(name="wnat", bufs=3))
        wbfp = ctx.enter_context(tc.tile_pool(name="wbf", bufs=2))
        xresp = ctx.enter_context(tc.tile_pool(name="xres", bufs=3))
        ostp = ctx.enter_context(tc.tile_pool(name="ost", bufs=2))
        psum_t = ctx.enter_context(tc.tile_pool(name="psum_t", bufs=3, space="PSUM"))
        psum_mm = ctx.enter_context(tc.tile_pool(name="psum_mm", bufs=5, space="PSUM"))

        # ---- identity first: it gates every PE transpose at kernel start ----
        id32 = consts.tile([P, P], F32, name="id32")
        make_identity(nc, id32)
        idb = consts.tile([P, P], BF16, name="idb")
        nc.vector.tensor_copy(out=idb, in_=id32)

        # persistent fp8 operand tensors
        WqT8 = wt8p.tile([P, DK, D], FP8, name="WqT8")   # (32 Wq)^T [d_in, dk, k]
        WkT8 = wt8p.tile([P, DK, D], FP8, name="WkT8")
        WvT8 = wt8p.tile([P, DK, D], FP8, name="WvT8")
        xT8 = xt8p.tile([P, DK, T], FP8, name="xT8")     # x^T [d_in, dk, t]
        KT8 = kt8p.tile([P, KO, T], FP8, name="KT8")     # (K+bk)^T [k_in, ko, s]
        Vp8 = vp8p.tile([P, SV, D], FP8, name="Vp8")     # 32(V+bv) [s_in, sv, v]
        Pbig = pbigp.tile([P, 40, TB], BF16, name="Pbig")  # exp(logits) [s_in, pb, t]
        Pq8 = [pq8p.tile([P, 4 * j + 4, TB], FP8, name=f"Pq8_{j}")
               for j in range(NTB)]                      # P/Z [s_in, sv, t] per j

        def dma_in_split(dst, src, nsplit=2):
            # alternate rings; split by partition to spread descriptors
            step = P // nsplit
            for q in range(nsplit):
                eng = nc.gpsimd if q % 2 == 0 else nc.sync
                eng.dma_start(out=dst[q * step:(q + 1) * step, :],
                              in_=src[q * step:(q + 1) * step, :])

        def psum_copy(idx, out_ap, in_ap):
            # spread PSUM->SBUF copies across vector/scalar (gpsimd stays
            # clean: its in-order queue issues the DMA load stream)
            if idx % 2 == 0:
                nc.vector.tensor_copy(out=out_ap, in_=in_ap)
            else:
                nc.scalar.activation(out_ap, in_ap, AF.Copy)

        def emit_x_block(j):
            """DMA x rows, convert bf16, transpose to xT8 fp8."""
            for ts_ in range(TB // P):
                t0 = j * TB + ts_ * P
                xnat = xnatp.tile([P, D], F32, name="xnat", tag="xnat")
                dma_in_split(xnat, x[t0:t0 + P, :], nsplit=4 if j == 0 else 2)
                xbf = xbfp.tile([P, D], BF16, name="xbf", tag="xbf")
                nc.vector.tensor_copy(out=xbf, in_=xnat)
                for dk in range(DK):
                    pt = psum_t.tile([P, P], BF16, name="pt", tag="pt")
                    nc.tensor.transpose(pt, xbf[:, dk * P:(dk + 1) * P], idb)
                    psum_copy(dk, xT8[:, dk, t0:t0 + P], pt)

        def emit_w(w_ap, dst):
            """DMA W rows, scale x32 into bf16, transpose to dst fp8."""
            for kt in range(8):
                wnat = wnatp.tile([P, D], F32, name="wnat", tag="wnat")
                dma_in_split(wnat, w_ap[kt * P:(kt + 1) * P, :])
                wbf = wbfp.tile([P, D], BF16, name="wbf", tag="wbf")
                nc.scalar.activation(wbf, wnat, AF.Copy, scale=WS)
                for dk in range(DK):
                    pt = psum_t.tile([P, P], BF16, name="pt", tag="pt")
                    nc.tensor.transpose(pt, wbf[:, dk * P:(dk + 1) * P], idb)
                    psum_copy(dk, dst[:, dk, kt * P:(kt + 1) * P], pt)

        # ---- x block 0 first (earliest PE start), then consts ----
        emit_x_block(0)

        # sliding staircase mask [128, 896]: 0 iff free >= part + 384 else -1e30
        mask_base = consts.tile([P, TB + 3 * P], BF16, name="mask_base")
        nc.gpsimd.memset(mask_base, 0.0)
        nc.gpsimd.affine_select(
            out=mask_base, in_=mask_base,
            compare_op=mybir.AluOpType.is_ge,
            fill=NEG,
            base=-(3 * P),
            pattern=[[1, TB + 3 * P]],
            channel_multiplier=-1,
        )
        masks = [mask_base[:, 3 * P - oi * P: 3 * P - oi * P + TB]
                 for oi in range(4)]

        # biases: bq/bk striped [128, 8] (per-partition, k-major);
        # bv broadcast to all partitions, scaled x32 to match Vp8
        bq_sb = consts.tile([P, KO], F32, name="bq_sb")
        nc.sync.dma_start(out=bq_sb, in_=bq.rearrange("(o p) -> p o", p=P))
        bk_sb = consts.tile([P, KO], F32, name="bk_sb")
        nc.sync.dma_start(out=bk_sb, in_=bk.rearrange("(o p) -> p o", p=P))
        bv_sb = consts.tile([P, D], F32, name="bv_sb")
        bv_bcast = bass.AP(tensor=bv.tensor, offset=bv.offset,
                           ap=[[0, P], [1, D]])
        nc.gpsimd.dma_start(out=bv_sb, in_=bv_bcast)
        bv32_sb = consts.tile([P, D], BF16, name="bv32_sb")
        nc.scalar.activation(bv32_sb, bv_sb, AF.Copy, scale=WS)

        Zacc = consts.tile([P, SV, NTB], F32, name="Zacc")
        nc.vector.memset(Zacc, 0.0)
        ztmp = consts.tile([P, SV], F32, name="ztmp")
        rtile = consts.tile([P, SV], F32, name="rtile")

        def emit_qkt(j, wt8, bias_sb, dst_ap_of_ko):
            """QT/KT block for t-block j: out[k 128, t 512] per ko, fp8."""
            for ko in range(KO):
                ps = psum_mm.tile([P, TB], F32, name="ps_qk", tag="mm")
                for a in range(4):
                    nc.tensor.matmul(
                        ps,
                        lhsT=wt8[:, 2 * a:2 * a + 2, ko * P:(ko + 1) * P],
                        rhs=xT8[:, 2 * a:2 * a + 2, j * TB:(j + 1) * TB],
                        start=(a == 0), stop=(a == 3),
                        perf_mode=DR,
                    )
                nc.scalar.activation(
                    dst_ap_of_ko(ko), ps, AF.Identity,
                    bias=bias_sb[:, ko:ko + 1], scale=1.0 / WS,
                )

        def emit_v(j):
            """V' tiles for s-blocks of j: Vp8[sv] = 32(V+bv) fp8."""
            for si in range(TB // P):
                sv = 4 * j + si
                s0 = sv * P
                for h in range(D // TB):
                    ps = psum_mm.tile([P, TB], F32, name="ps_v", tag="mm")
                    for a in range(4):
                        nc.tensor.matmul(
                            ps,
                            lhsT=xT8[:, 2 * a:2 * a + 2, s0:s0 + P],
                            rhs=WvT8[:, 2 * a:2 * a + 2, h * TB:(h + 1) * TB],
                            start=(a == 0), stop=(a == 3),
                            perf_mode=DR,
                        )
                    nc.vector.tensor_add(
                        out=Vp8[:, sv, h * TB:(h + 1) * TB],
                        in0=ps, in1=bv32_sb[:, h * TB:(h + 1) * TB],
                    )

        def emit_logits_exp(j, sv, qt8):
            """logits tile [s 128, t 512] -> exp -> Pbig; Z accum."""
            ps = psum_mm.tile([P, TB], F32, name="ps_l", tag="mm")
            for a in range(4):
                nc.tensor.matmul(
                    ps,
                    lhsT=KT8[:, 2 * a:2 * a + 2, sv * P:(sv + 1) * P],
                    rhs=qt8[:, 2 * a:2 * a + 2, :],
                    start=(a == 0), stop=(a == 3),
                    perf_mode=DR,
                )
            oi = sv - 4 * j
            if oi >= 0:
                nc.vector.tensor_add(out=ps, in0=ps, in1=masks[oi])
            nc.scalar.activation(
                Pbig[:, PBASE[j] + sv, :], ps, AF.Exp,
                accum_out=Zacc[:, sv, j:j + 1],
            )

        def emit_out_tile(i):
            """out rows [i*128, (i+1)*128): PV fp8 DR + 1/1024 scale + resid."""
            jj = i // 4
            tc_ = i % 4
            xres = xresp.tile([P, D], F32, name="xres", tag="xres")
            nc.gpsimd.dma_start(out=xres, in_=x[i * P:(i + 1) * P, :])
            ost = ostp.tile([P, D], F32, name="ost", tag="ost")
            npair = (i + 2) // 2
            for h in range(D // TB):
                ps = psum_mm.tile([P, TB], F32, name="ps_o", tag="mm")
                for a in range(npair):
                    nc.tensor.matmul(
                        ps,
                        lhsT=Pq8[jj][:, 2 * a:2 * a + 2, tc_ * P:(tc_ + 1) * P],
                        rhs=Vp8[:, 2 * a:2 * a + 2, h * TB:(h + 1) * TB],
                        start=(a == 0), stop=(a == npair - 1),
                        perf_mode=DR,
                    )
                nc.scalar.activation(
                    ost[:, h * TB:(h + 1) * TB], ps, AF.Copy,
                    scale=1.0 / (WS * WS),
                )
                nc.vector.tensor_add(
                    out=ost[:, h * TB:(h + 1) * TB],
                    in0=ost[:, h * TB:(h + 1) * TB],
                    in1=xres[:, h * TB:(h + 1) * TB],
                )
            nc.sync.dma_start(out=out[i * P:(i + 1) * P, :], in_=ost)

        # ---- main pipeline ----
        for j in range(NTB):
            if j > 0:
                emit_x_block(j)
            qt8 = qt8p.tile([P, KO, TB], FP8, name="qt8", tag="qt8")
            if j == 0:
                emit_w(Wq, WqT8)
            emit_qkt(j, WqT8, bq_sb, lambda ko: qt8[:, ko, :])
            if j == 0:
                emit_w(Wk, WkT8)
            emit_qkt(j, WkT8, bk_sb,
                     lambda ko: KT8[:, ko, j * TB:(j + 1) * TB])
            if j == 0:
                emit_w(Wv, WvT8)
            emit_v(j)

            for sv in range(4 * (j + 1)):
                emit_logits_exp(j, sv, qt8)
                if j == NTB - 1:
                    # Z[sv] is final: R = 1/Z, then normalize+convert the sv
                    # column of every j' block to fp8, then emit out-tile sv-1
                    nc.vector.reduce_sum(out=ztmp[:, sv:sv + 1],
                                         in_=Zacc[:, sv, :],
                                         axis=mybir.AxisListType.X)
                    nc.vector.reciprocal(rtile[:, sv:sv + 1],
                                         ztmp[:, sv:sv + 1])
                    for jp in range(NTB):
                        if sv <= 4 * jp + 3:
                            if (jp + sv) % 2 == 0:
                                nc.vector.tensor_scalar_mul(
                                    Pq8[jp][:, sv, :],
                                    Pbig[:, PBASE[jp] + sv, :],
                                    rtile[:, sv:sv + 1],
                                )
                            else:
                                nc.scalar.activation(
                                    Pq8[jp][:, sv, :],
                                    Pbig[:, PBASE[jp] + sv, :],
                                    AF.Identity, scale=rtile[:, sv:sv + 1],
                                )
                    if sv >= 1:
                        emit_out_tile(sv - 1)
        emit_out_tile(SV - 1)


_NC_CACHE = None


def _get_nc():
    global _NC_CACHE
    if _NC_CACHE is None:
        _NC_CACHE = _build_nc()
    return _NC_CACHE


def kernel(minibatch, Wq, bq, Wk, bk, Wv, bv):
    minibatch = np.asarray(minibatch, dtype=np.float32)
    Wq = np.asarray(Wq, dtype=np.float32)
    bq = np.asarray(bq, dtype=np.float32)
    Wk = np.asarray(Wk, dtype=np.float32)
    bk = np.asarray(bk, dtype=np.float32)
    Wv = np.asarray(Wv, dtype=np.float32)
    bv = np.asarray(bv, dtype=np.float32)

    nc = _get_nc()
    B = minibatch.shape[0]
    in_maps = [
        {
            "x": np.ascontiguousarray(minibatch[i]),
            "Wq": Wq, "bq": bq, "Wk": Wk, "bk": bk, "Wv": Wv, "bv": bv,
        }
        for i in range(B)
    ]
    last_err = None
    for _attempt in range(3):
        try:
            res = run_bass_kernel_spmd(nc, in_maps, core_ids=list(range(B)))
            break
        except Exception as e:  # transient device errors
            last_err = e
            time.sleep(2.0)
    else:
        raise last_err
    return np.stack([res.results[i]["out"] for i in range(B)], axis=0)
